# revision 14
# baseline (speedup 1.0000x reference)
"""ArgmaxIOU kernel for 8 Trainium2 NeuronCores.

Data-parallel over batch: core i processes sample i (shapes hardcoded:
B=8, C=21, H=W=512). Raw Bass (explicit engines + semaphores).

Per pixel tile (columns of the per-partition pixel range), per core:
  gpsimd: SWDGE DMA loads with inline f32->bf16 cast (halves SBUF traffic,
          removes the convert stage entirely)
  DVE:    pairwise-max tree over the 21 classes (bf16 2x mode), then
          is_equal against the broadcast max -> one-hot argmax mask,
          scattered into the G-interleaved matmul layout [TB, C, G]
  PE:     matmul eqt^T @ eqp per G-block, accumulating a packed [84, 84]
          confusion matrix in PSUM: conf[c,c'] = sum_g out[4c+g, 4c'+g]

The tile schedule starts and ends with small tiles to shorten pipeline
fill/drain. Host: gather the 8 packed matrices, fold G, compute mean IoU.
bf16 argmax quantization shifts the score by ~5e-4 relative (bf16 argmax
ties are rare and wash out of the large confusion counts).
"""

import sys

import numpy as np

for p in ("/opt/trn_rl_repo",):
    if p not in sys.path:
        sys.path.insert(0, p)

from concourse import bass, mybir
from concourse.bass_utils import run_bass_kernel_spmd

B = 8
C = 21
HW = 512 * 512
P = 128
Q = HW // P              # pixels per partition (2048)
G = 4                    # t-columns packed per matmul (4*21=84 <= 128)
M = G * C                # 84
TMAX = 384
NSLOT = 3                # bf16 data tile slots (DMA runs ahead)
TS = [64, 64, 128, 256, 384, 384, 256, 256, 128, 64, 64]   # sums to 2048
assert sum(TS) == Q
J = len(TS)

F32 = mybir.dt.float32
BF16 = mybir.dt.bfloat16


def build():
    nc = bass.Bass(dynamic_dma_scratch_size=32768)
    pred = nc.declare_dram_parameter("prediction", [C, HW], F32, isOutput=False)
    targ = nc.declare_dram_parameter("target", [C, HW], F32, isOutput=False)
    out = nc.declare_dram_parameter("out", [M, M], F32, isOutput=True)

    # partition p owns pixels [p*Q, (p+1)*Q); tile j covers columns
    # off_j .. off_j+T_j of every partition
    predv = pred[:].rearrange("c (p q) -> p c q", p=P)
    targv = targ[:].rearrange("c (p q) -> p c q", p=P)

    offs = []
    o = 0
    for t in TS:
        offs.append(o)
        o += t

    mx = mybir.AluOpType.max
    eqop = mybir.AluOpType.is_equal

    with (
        nc.sbuf_tensor("bt", [P, NSLOT, C, TMAX], BF16) as bt,
        nc.sbuf_tensor("bp", [P, NSLOT, C, TMAX], BF16) as bp,
        nc.sbuf_tensor("eqt", [P, 2, TMAX // G, C, G], BF16) as eqt,
        nc.sbuf_tensor("eqp", [P, 2, TMAX // G, C, G], BF16) as eqp,
        nc.sbuf_tensor("st", [P, 10, TMAX], BF16) as st,
        nc.sbuf_tensor("sp", [P, 10, TMAX], BF16) as sp,
        nc.sbuf_tensor("mt", [P, TMAX], BF16) as mt,
        nc.sbuf_tensor("mp", [P, TMAX], BF16) as mp,
        nc.sbuf_tensor("osb", [M, M], F32) as osb,
        nc.psum_tensor("conf", [M, M], F32) as conf,
        nc.semaphore("dml") as dml,
        nc.semaphore("dve") as dve,
        nc.semaphore("mm") as mm,
        nc.semaphore("fin") as fin,
        nc.Block() as block,
    ):

        @block.gpsimd
        def _(g):
            for j in range(J):
                s = j % NSLOT
                off, t = offs[j], TS[j]
                if j >= NSLOT:
                    g.wait_ge(dve, 2 * (j - NSLOT) + 2)  # DVE done with slot
                g.dma_start(out=bt[:, s, :, 0:t],
                            in_=targv[:, :, off:off + t]).then_inc(dml, 16)
                g.dma_start(out=bp[:, s, :, 0:t],
                            in_=predv[:, :, off:off + t]).then_inc(dml, 16)
            g.wait_ge(dve, 2 * J + 1)                    # osb written
            g.dma_start(out=out[:], in_=osb[:]).then_inc(fin, 16)
            g.wait_ge(fin, 16)

        @block.vector
        def _(v):
            def tree(data, scr, mo, t):
                # max over 21 class slices of data [P, C, t] -> mo [P, t]
                v.tensor_tensor(scr[:, 0:10, 0:t], data[:, 0:10, 0:t],
                                data[:, 10:20, 0:t], mx)
                v.tensor_tensor(scr[:, 0:5, 0:t], scr[:, 0:5, 0:t],
                                scr[:, 5:10, 0:t], mx)
                v.tensor_tensor(scr[:, 0:2, 0:t], scr[:, 0:2, 0:t],
                                scr[:, 2:4, 0:t], mx)
                v.tensor_tensor(scr[:, 0:1, 0:t], scr[:, 0:1, 0:t],
                                scr[:, 1:2, 0:t], mx)
                v.tensor_tensor(scr[:, 0:1, 0:t], scr[:, 0:1, 0:t],
                                scr[:, 4:5, 0:t], mx)
                v.tensor_tensor(mo[:, 0:t].unsqueeze(1), scr[:, 0:1, 0:t],
                                data[:, 20:21, 0:t], mx)

            for j in range(J):
                s = j % NSLOT
                e = j % 2
                off, t = offs[j], TS[j]
                tb = t // G
                v.wait_ge(dml, 32 * j + 16)
                tree(bt[:, s], st, mt, t)
                if j >= 2:
                    v.wait_ge(mm, j - 1)                 # PE done with eq slot
                v.tensor_tensor(
                    eqt[:, e, 0:tb].rearrange("p tb c g -> p c tb g"),
                    bt[:, s, :, 0:t].rearrange("p c (tb g) -> p c tb g", g=G),
                    mt[:, 0:t].rearrange("p (tb g) -> p tb g", g=G)
                        .unsqueeze(1).broadcast_to((P, C, tb, G)),
                    eqop).then_inc(dve, 1)
                v.wait_ge(dml, 32 * j + 32)
                tree(bp[:, s], sp, mp, t)
                v.tensor_tensor(
                    eqp[:, e, 0:tb].rearrange("p tb c g -> p c tb g"),
                    bp[:, s, :, 0:t].rearrange("p c (tb g) -> p c tb g", g=G),
                    mp[:, 0:t].rearrange("p (tb g) -> p tb g", g=G)
                        .unsqueeze(1).broadcast_to((P, C, tb, G)),
                    eqop).then_inc(dve, 1)
            v.wait_ge(mm, J)
            v.tensor_copy(osb[:], conf[:]).then_inc(dve, 1)

        @block.tensor
        def _(te):
            for j in range(J):
                e = j % 2
                tbs = TS[j] // G
                te.wait_ge(dve, 2 * j + 2)
                for tb in range(tbs):
                    inst = te.matmul(
                        conf[:],
                        eqt[:, e, tb].rearrange("p c g -> p (c g)"),
                        eqp[:, e, tb].rearrange("p c g -> p (c g)"),
                        start=(j == 0 and tb == 0),
                        stop=(j == J - 1 and tb == tbs - 1))
                    if tb == tbs - 1:
                        inst.then_inc(mm, 1)

    return nc


def _score_from_packed(packed):
    """packed: [84, 84] f32 -> per-sample mean IoU (float64)."""
    x = packed.astype(np.float64).reshape(C, G, C, G)
    conf = np.einsum("igjg->ij", x)
    TP = np.diag(conf).copy()
    FN = conf.sum(axis=1) - TP
    FP = conf.sum(axis=0) - TP
    valid = TP > 0
    denom = TP + FN + FP
    iou = np.where(valid, TP / np.where(valid, denom, 1.0), 0.0)
    n_valid = max(float(valid.sum()), 1.0)
    return iou.sum() / n_valid


_NC_CACHE = {}


def _get_nc():
    if "nc" not in _NC_CACHE:
        _NC_CACHE["nc"] = build()
    return _NC_CACHE["nc"]


def run(prediction, target, trace=False):
    in_maps = []
    for i in range(B):
        in_maps.append({
            "prediction": np.ascontiguousarray(
                np.asarray(prediction[i], dtype=np.float32).reshape(C, HW)),
            "target": np.ascontiguousarray(
                np.asarray(target[i], dtype=np.float32).reshape(C, HW)),
        })
    res = run_bass_kernel_spmd(_get_nc(), in_maps, core_ids=list(range(B)),
                               trace=trace)
    scores = [_score_from_packed(res.results[i]["out"]) for i in range(B)]
    return np.float32(np.mean(scores)), res


def kernel(prediction, target):
    score, _ = run(prediction, target, trace=False)
    return score


# revision 15
# speedup vs baseline: 1.0154x; 1.0154x over previous
"""ArgmaxIOU kernel for 8 Trainium2 NeuronCores.

Data-parallel over batch: core i processes sample i (shapes hardcoded:
B=8, C=21, H=W=512). Raw Bass (explicit engines + semaphores).

Per pixel tile (columns of the per-partition pixel range), per core:
  gpsimd: SWDGE DMA loads with inline f32->bf16 cast (halves SBUF traffic,
          removes the convert stage entirely)
  DVE:    pairwise-max tree over the 21 classes (bf16 2x mode), then
          is_equal against the broadcast max -> one-hot argmax mask,
          scattered into the G-interleaved matmul layout [TB, C, G]
  PE:     matmul eqt^T @ eqp per G-block, accumulating a packed [84, 84]
          confusion matrix in PSUM: conf[c,c'] = sum_g out[4c+g, 4c'+g]

The tile schedule starts and ends with small tiles to shorten pipeline
fill/drain. Host: gather the 8 packed matrices, fold G, compute mean IoU.
bf16 argmax quantization shifts the score by ~5e-4 relative (bf16 argmax
ties are rare and wash out of the large confusion counts).
"""

import sys

import numpy as np

for p in ("/opt/trn_rl_repo",):
    if p not in sys.path:
        sys.path.insert(0, p)

from concourse import bass, mybir
from concourse.bass_utils import run_bass_kernel_spmd

B = 8
C = 21
HW = 512 * 512
P = 128
Q = HW // P              # pixels per partition (2048)
G = 4                    # t-columns packed per matmul (4*21=84 <= 128)
M = G * C                # 84
TMAX = 256
NSLOT = 4                # bf16 data tile slots (DMA runs ahead)
NEQ = 3                  # one-hot mask slots
TS = [64, 64, 128] + [256] * 6 + [128, 64, 64]   # sums to 2048
assert sum(TS) == Q
J = len(TS)

F32 = mybir.dt.float32
BF16 = mybir.dt.bfloat16


def build():
    nc = bass.Bass()
    pred = nc.declare_dram_parameter("prediction", [C, HW], F32, isOutput=False)
    targ = nc.declare_dram_parameter("target", [C, HW], F32, isOutput=False)
    out = nc.declare_dram_parameter("out", [M, M], F32, isOutput=True)

    # partition p owns pixels [p*Q, (p+1)*Q); tile j covers columns
    # off_j .. off_j+T_j of every partition
    predv = pred[:].rearrange("c (p q) -> p c q", p=P)
    targv = targ[:].rearrange("c (p q) -> p c q", p=P)

    offs = []
    o = 0
    for t in TS:
        offs.append(o)
        o += t

    mx = mybir.AluOpType.max
    eqop = mybir.AluOpType.is_equal

    with (
        nc.sbuf_tensor("bt", [P, NSLOT, C, TMAX], BF16) as bt,
        nc.sbuf_tensor("bp", [P, NSLOT, C, TMAX], BF16) as bp,
        nc.sbuf_tensor("eqt", [P, NEQ, TMAX // G, C, G], BF16) as eqt,
        nc.sbuf_tensor("eqp", [P, NEQ, TMAX // G, C, G], BF16) as eqp,
        nc.sbuf_tensor("st", [P, 10, TMAX], BF16) as st,
        nc.sbuf_tensor("sp", [P, 10, TMAX], BF16) as sp,
        nc.sbuf_tensor("mt", [P, TMAX], BF16) as mt,
        nc.sbuf_tensor("mp", [P, TMAX], BF16) as mp,
        nc.sbuf_tensor("osb", [M, M], F32) as osb,
        nc.psum_tensor("conf", [M, M], F32) as conf,
        nc.semaphore("dml") as dml,
        nc.semaphore("dve") as dve,
        nc.semaphore("mm") as mm,
        nc.semaphore("fin") as fin,
        nc.Block() as block,
    ):

        @block.gpsimd
        def _(g):
            for j in range(J):
                s = j % NSLOT
                off, t = offs[j], TS[j]
                if j >= NSLOT:
                    g.wait_ge(dve, 2 * (j - NSLOT) + 2)  # DVE done with slot
                g.dma_start(out=bt[:, s, :, 0:t],
                            in_=targv[:, :, off:off + t]).then_inc(dml, 16)
                g.dma_start(out=bp[:, s, :, 0:t],
                            in_=predv[:, :, off:off + t]).then_inc(dml, 16)
            g.wait_ge(dve, 2 * J + 1)                    # osb written
            g.dma_start(out=out[:], in_=osb[:]).then_inc(fin, 16)
            g.wait_ge(fin, 16)

        @block.vector
        def _(v):
            def tree(data, scr, mo, t):
                # max over 21 class slices of data [P, C, t] -> mo [P, t]
                v.tensor_tensor(scr[:, 0:10, 0:t], data[:, 0:10, 0:t],
                                data[:, 10:20, 0:t], mx)
                v.tensor_tensor(scr[:, 0:5, 0:t], scr[:, 0:5, 0:t],
                                scr[:, 5:10, 0:t], mx)
                v.tensor_tensor(scr[:, 0:2, 0:t], scr[:, 0:2, 0:t],
                                scr[:, 2:4, 0:t], mx)
                v.tensor_tensor(scr[:, 0:1, 0:t], scr[:, 0:1, 0:t],
                                scr[:, 1:2, 0:t], mx)
                v.tensor_tensor(scr[:, 0:1, 0:t], scr[:, 0:1, 0:t],
                                scr[:, 4:5, 0:t], mx)
                v.tensor_tensor(mo[:, 0:t].unsqueeze(1), scr[:, 0:1, 0:t],
                                data[:, 20:21, 0:t], mx)

            for j in range(J):
                s = j % NSLOT
                e = j % NEQ
                off, t = offs[j], TS[j]
                tb = t // G
                v.wait_ge(dml, 32 * j + 16)
                tree(bt[:, s], st, mt, t)
                if j >= NEQ:
                    v.wait_ge(mm, j - NEQ + 1)           # PE done with eq slot
                v.tensor_tensor(
                    eqt[:, e, 0:tb].rearrange("p tb c g -> p c tb g"),
                    bt[:, s, :, 0:t].rearrange("p c (tb g) -> p c tb g", g=G),
                    mt[:, 0:t].rearrange("p (tb g) -> p tb g", g=G)
                        .unsqueeze(1).broadcast_to((P, C, tb, G)),
                    eqop).then_inc(dve, 1)
                v.wait_ge(dml, 32 * j + 32)
                tree(bp[:, s], sp, mp, t)
                v.tensor_tensor(
                    eqp[:, e, 0:tb].rearrange("p tb c g -> p c tb g"),
                    bp[:, s, :, 0:t].rearrange("p c (tb g) -> p c tb g", g=G),
                    mp[:, 0:t].rearrange("p (tb g) -> p tb g", g=G)
                        .unsqueeze(1).broadcast_to((P, C, tb, G)),
                    eqop).then_inc(dve, 1)
            v.wait_ge(mm, J)
            v.tensor_copy(osb[:], conf[:]).then_inc(dve, 1)

        @block.tensor
        def _(te):
            for j in range(J):
                e = j % NEQ
                tbs = TS[j] // G
                te.wait_ge(dve, 2 * j + 2)
                for tb in range(tbs):
                    inst = te.matmul(
                        conf[:],
                        eqt[:, e, tb].rearrange("p c g -> p (c g)"),
                        eqp[:, e, tb].rearrange("p c g -> p (c g)"),
                        start=(j == 0 and tb == 0),
                        stop=(j == J - 1 and tb == tbs - 1))
                    if tb == tbs - 1:
                        inst.then_inc(mm, 1)

    return nc


def _score_from_packed(packed):
    """packed: [84, 84] f32 -> per-sample mean IoU (float64)."""
    x = packed.astype(np.float64).reshape(C, G, C, G)
    conf = np.einsum("igjg->ij", x)
    TP = np.diag(conf).copy()
    FN = conf.sum(axis=1) - TP
    FP = conf.sum(axis=0) - TP
    valid = TP > 0
    denom = TP + FN + FP
    iou = np.where(valid, TP / np.where(valid, denom, 1.0), 0.0)
    n_valid = max(float(valid.sum()), 1.0)
    return iou.sum() / n_valid


_NC_CACHE = {}


def _get_nc():
    if "nc" not in _NC_CACHE:
        _NC_CACHE["nc"] = build()
    return _NC_CACHE["nc"]


def run(prediction, target, trace=False):
    in_maps = []
    for i in range(B):
        in_maps.append({
            "prediction": np.ascontiguousarray(
                np.asarray(prediction[i], dtype=np.float32).reshape(C, HW)),
            "target": np.ascontiguousarray(
                np.asarray(target[i], dtype=np.float32).reshape(C, HW)),
        })
    res = run_bass_kernel_spmd(_get_nc(), in_maps, core_ids=list(range(B)),
                               trace=trace)
    scores = [_score_from_packed(res.results[i]["out"]) for i in range(B)]
    return np.float32(np.mean(scores)), res


def kernel(prediction, target):
    score, _ = run(prediction, target, trace=False)
    return score


# revision 22
# speedup vs baseline: 1.0290x; 1.0135x over previous
"""ArgmaxIOU kernel for 8 Trainium2 NeuronCores.

Data-parallel over batch: core i processes sample i (shapes hardcoded:
B=8, C=21, H=W=512). Raw Bass (explicit engines + semaphores).

Per 128x(21x256) pixel tile, per core:
  gpsimd: SWDGE DMA loads with inline f32->bf16 cast (halves SBUF traffic,
          no separate convert stage)
  DVE:    pairwise-max tree over the 21 classes (bf16 2x mode), then
          is_equal against the broadcast max -> one-hot argmax mask,
          scattered into the G-interleaved matmul layout [TB, C, G]
  PE:     matmul eqt^T @ eqp per G-block, accumulating a packed [84, 84]
          confusion matrix in PSUM: conf[c,c'] = sum_g out[4c+g, 4c'+g]
  ACT:    final PSUM -> SBUF extraction (otherwise idle)

Tile 0's loads are split by class range (10 + 11 classes, full 128
partitions, >=1 KB runs) so the first max-tree starts after roughly half
the first transfer — shortens pipeline fill by ~10 us.

Host: gather the 8 packed matrices, fold G, compute mean IoU. bf16 argmax
quantization shifts the score by ~5e-4 relative (bf16 ties are rare and
wash out of the large confusion counts).

Determinism notes (hardware-verified):
 - one DMA-completion semaphore per in-flight load (round-robin pool):
   summing all loads on one semaphore is racy across the 16 SDMA engines
 - uniform 256-column tiles only: sub-256-column tiles produced
   nondeterministic DMA completion behavior on this SWDGE cast path
"""

import sys

import numpy as np

for p in ("/opt/trn_rl_repo",):
    if p not in sys.path:
        sys.path.insert(0, p)

from contextlib import ExitStack

from concourse import bass, mybir
from concourse.bass_utils import run_bass_kernel_spmd

B = 8
C = 21
HW = 512 * 512
P = 128
Q = HW // P              # pixels per partition (2048)
G = 4                    # t-columns packed per matmul (4*21=84 <= 128)
M = G * C                # 84
T = 256                  # pixels per partition per tile
J = Q // T               # 8 tiles
TB = T // G              # 64 matmul blocks per tile
NSLOT = 4                # bf16 data tile slots (DMA runs ahead)
NEQ = 3                  # one-hot mask slots
NDM = 12                 # DMA-completion semaphore pool

F32 = mybir.dt.float32
BF16 = mybir.dt.bfloat16


def build():
    nc = bass.Bass()
    pred = nc.declare_dram_parameter("prediction", [C, HW], F32, isOutput=False)
    targ = nc.declare_dram_parameter("target", [C, HW], F32, isOutput=False)
    out = nc.declare_dram_parameter("out", [M, M], F32, isOutput=True)

    # partition p owns pixels [p*Q, (p+1)*Q); tile j covers columns
    # [j*T, (j+1)*T) of every partition
    predv = pred[:].rearrange("c (p q) -> p c q", p=P)
    targv = targ[:].rearrange("c (p q) -> p c q", p=P)

    mx = mybir.AluOpType.max
    eqop = mybir.AluOpType.is_equal
    cp = mybir.ActivationFunctionType.Copy

    # load ledger: tile 0 is class-split (0:10 / 10:21) per tensor; the
    # rest load all 21 classes at once. Issue order == list order.
    loads = []              # (j, tensor_id, c_lo, c_hi)
    for j in range(J):
        if j == 0:
            loads += [(j, 0, 0, 10), (j, 0, 10, 21),
                      (j, 1, 0, 10), (j, 1, 10, 21)]
        else:
            loads += [(j, 0, 0, 21), (j, 1, 0, 21)]
    lidx = {key: i for i, key in enumerate(loads)}

    def dm_of(key):
        i = lidx[key]
        return i, 16 * (i // NDM + 1)

    with ExitStack() as ctx:
        e_ = ctx.enter_context
        bt = e_(nc.sbuf_tensor("bt", [P, NSLOT, C, T], BF16))
        bp = e_(nc.sbuf_tensor("bp", [P, NSLOT, C, T], BF16))
        eqt = e_(nc.sbuf_tensor("eqt", [P, NEQ, TB, C, G], BF16))
        eqp = e_(nc.sbuf_tensor("eqp", [P, NEQ, TB, C, G], BF16))
        st = e_(nc.sbuf_tensor("st", [P, 10, T], BF16))
        sp = e_(nc.sbuf_tensor("sp", [P, 10, T], BF16))
        mt = e_(nc.sbuf_tensor("mt", [P, T], BF16))
        mp = e_(nc.sbuf_tensor("mp", [P, T], BF16))
        osb = e_(nc.sbuf_tensor("osb", [M, M], F32))
        conf = e_(nc.psum_tensor("conf", [M, M], F32))
        dms = [e_(nc.semaphore(f"dm{i}")) for i in range(NDM)]
        dve = e_(nc.semaphore("dve"))
        mm = e_(nc.semaphore("mm"))
        fin = e_(nc.semaphore("fin"))
        block = e_(nc.Block())

        srcs = {0: targv, 1: predv}
        dsts = {0: bt, 1: bp}

        @block.gpsimd
        def _(g):
            cur = -1
            for (j, tid, clo, chi) in loads:
                if j != cur:
                    cur = j
                    if j >= NSLOT:
                        g.wait_ge(dve, 2 * (j - NSLOT) + 2)
                s = j % NSLOT
                i, _ = dm_of((j, tid, clo, chi))
                g.dma_start(
                    out=dsts[tid][:, s, clo:chi, :],
                    in_=srcs[tid][:, clo:chi, j * T:(j + 1) * T],
                ).then_inc(dms[i % NDM], 16)
            g.wait_ge(dve, 2 * J + 1)                    # osb written (ACT)
            g.dma_start(out=out[:], in_=osb[:]).then_inc(fin, 16)
            g.wait_ge(fin, 16)

        @block.vector
        def _(v):
            def w(key):
                i, val = dm_of(key)
                v.wait_ge(dms[i % NDM], val)

            def tree21(data, scr, mo):
                # max over all 21 class slices -> mo [P, T]
                v.tensor_tensor(scr[:, 0:10, :], data[:, 0:10, :],
                                data[:, 10:20, :], mx)
                v.tensor_tensor(scr[:, 0:5, :], scr[:, 0:5, :],
                                scr[:, 5:10, :], mx)
                v.tensor_tensor(scr[:, 0:2, :], scr[:, 0:2, :],
                                scr[:, 2:4, :], mx)
                v.tensor_tensor(scr[:, 0:1, :], scr[:, 0:1, :],
                                scr[:, 1:2, :], mx)
                v.tensor_tensor(scr[:, 0:1, :], scr[:, 0:1, :],
                                scr[:, 4:5, :], mx)
                v.tensor_tensor(mo[:].unsqueeze(1), scr[:, 0:1, :],
                                data[:, 20:21, :], mx)

            def treeA(data, scr):
                # partial max over classes 0:10 -> scr[:, 0:1, :]
                v.tensor_tensor(scr[:, 0:5, :], data[:, 0:5, :],
                                data[:, 5:10, :], mx)
                v.tensor_tensor(scr[:, 0:2, :], scr[:, 0:2, :],
                                scr[:, 2:4, :], mx)
                v.tensor_tensor(scr[:, 0:1, :], scr[:, 0:1, :],
                                scr[:, 1:2, :], mx)
                v.tensor_tensor(scr[:, 0:1, :], scr[:, 0:1, :],
                                scr[:, 4:5, :], mx)

            def treeB(data, scr, mo):
                # partial max over classes 10:21 -> fold with scr[:, 0:1, :]
                v.tensor_tensor(scr[:, 5:10, :], data[:, 10:15, :],
                                data[:, 15:20, :], mx)
                v.tensor_tensor(scr[:, 5:7, :], scr[:, 5:7, :],
                                scr[:, 7:9, :], mx)
                v.tensor_tensor(scr[:, 5:6, :], scr[:, 5:6, :],
                                scr[:, 6:7, :], mx)
                v.tensor_tensor(scr[:, 5:6, :], scr[:, 5:6, :],
                                scr[:, 9:10, :], mx)
                v.tensor_tensor(scr[:, 5:6, :], scr[:, 5:6, :],
                                data[:, 20:21, :], mx)
                v.tensor_tensor(mo[:].unsqueeze(1), scr[:, 0:1, :],
                                scr[:, 5:6, :], mx)

            def eq(data, mo, dst, e):
                v.tensor_tensor(
                    dst[:, e].rearrange("p tb c g -> p c tb g"),
                    data[:].rearrange("p c (tb g) -> p c tb g", g=G),
                    mo[:].rearrange("p (tb g) -> p tb g", g=G)
                        .unsqueeze(1).broadcast_to((P, C, TB, G)),
                    eqop).then_inc(dve, 1)

            for j in range(J):
                s = j % NSLOT
                e = j % NEQ
                if j >= NEQ:
                    v.wait_ge(mm, j - NEQ + 1)           # PE done with eq slot
                if j == 0:
                    w((j, 0, 0, 10))
                    treeA(bt[:, s], st)
                    w((j, 0, 10, 21))
                    treeB(bt[:, s], st, mt)
                    eq(bt[:, s], mt, eqt, e)
                    w((j, 1, 0, 10))
                    treeA(bp[:, s], sp)
                    w((j, 1, 10, 21))
                    treeB(bp[:, s], sp, mp)
                    eq(bp[:, s], mp, eqp, e)
                else:
                    w((j, 0, 0, 21))
                    tree21(bt[:, s], st, mt)
                    eq(bt[:, s], mt, eqt, e)
                    w((j, 1, 0, 21))
                    tree21(bp[:, s], sp, mp)
                    eq(bp[:, s], mp, eqp, e)

        @block.scalar
        def _(sc):
            sc.wait_ge(mm, J)
            sc.activation(osb[:], conf[:], cp).then_inc(dve, 1)

        @block.tensor
        def _(te):
            for j in range(J):
                e = j % NEQ
                te.wait_ge(dve, 2 * j + 2)
                for tb in range(TB):
                    inst = te.matmul(
                        conf[:],
                        eqt[:, e, tb].rearrange("p c g -> p (c g)"),
                        eqp[:, e, tb].rearrange("p c g -> p (c g)"),
                        start=(j == 0 and tb == 0),
                        stop=(j == J - 1 and tb == TB - 1))
                    if tb == TB - 1:
                        inst.then_inc(mm, 1)

    return nc


def _score_from_packed(packed):
    """packed: [84, 84] f32 -> per-sample mean IoU (float64)."""
    x = packed.astype(np.float64).reshape(C, G, C, G)
    conf = np.einsum("igjg->ij", x)
    TP = np.diag(conf).copy()
    FN = conf.sum(axis=1) - TP
    FP = conf.sum(axis=0) - TP
    valid = TP > 0
    denom = TP + FN + FP
    iou = np.where(valid, TP / np.where(valid, denom, 1.0), 0.0)
    n_valid = max(float(valid.sum()), 1.0)
    return iou.sum() / n_valid


_NC_CACHE = {}


def _get_nc():
    if "nc" not in _NC_CACHE:
        _NC_CACHE["nc"] = build()
    return _NC_CACHE["nc"]


def run(prediction, target, trace=False):
    in_maps = []
    for i in range(B):
        in_maps.append({
            "prediction": np.ascontiguousarray(
                np.asarray(prediction[i], dtype=np.float32).reshape(C, HW)),
            "target": np.ascontiguousarray(
                np.asarray(target[i], dtype=np.float32).reshape(C, HW)),
        })
    res = run_bass_kernel_spmd(_get_nc(), in_maps, core_ids=list(range(B)),
                               trace=trace)
    scores = [_score_from_packed(res.results[i]["out"]) for i in range(B)]
    return np.float32(np.mean(scores)), res


def kernel(prediction, target):
    score, _ = run(prediction, target, trace=False)
    return score
